# revision 3
# baseline (speedup 1.0000x reference)
"""Trainium2 Bass kernel for nn_GATNet (2-layer multi-head GAT).

Sharding: data-parallel over batch B=16 across 8 cores (2 batches/core);
graph and weights replicated. No collectives.

Per (batch, head): scores s = h@h.T are computed with split-fp16 (3 K-passes,
~fp32 accuracy at full PE rate); the adjacency mask is pre-added into PSUM via
an identity matmul of gm = (graph-1)*1e16; row-softmax uses a DVE
reduce_max(negate) + one ACT Exp pass (per-partition bias, fused row-sum);
attention matrices are transposed via the DMA xbar (fp16) for the a@h matmul,
which runs as a 2-pass fp16 split to keep h at ~fp32 precision (layer-2 scores
amplify any error in the layer-1 output by ~1e6).
"""
import json
import sys

import numpy as np

sys.path.insert(0, "/opt/trn_rl_repo")

B, N, C_IN, HID, HEADS, C_OUT = 16, 1024, 64, 128, 8, 64
N_CORES = 8
B_PER_CORE = B // N_CORES
NBLK = N // 128  # 8


def _split_waits_json(bir_bytes: bytes, max_waits: int = 1) -> bytes:
    """walrus in this toolchain accepts only one sync-wait per instruction;
    hoist extra waits onto preceding same-engine NoOps."""
    d = json.loads(bir_bytes)
    for fn in d.get("functions", []):
        for bb in fn.get("blocks", []):
            out = []
            for inst in bb.get("instructions", []):
                si = inst.get("sync_info") or {}
                waits = si.get("on_wait") or []
                if len(waits) > max_waits:
                    extra, si["on_wait"] = waits[:-max_waits], waits[-max_waits:]
                    for k in range(0, len(extra), max_waits):
                        out.append({
                            "opcode": "NoOp",
                            "name": f"{inst['name']}-wsplit{k}",
                            "engine": inst["engine"],
                            "ins": [],
                            "outs": [],
                            "sync_info": {"on_wait": extra[k:k + max_waits],
                                          "on_update": []},
                        })
                out.append(inst)
            bb["instructions"] = out
    return json.dumps(d).encode()


def _f16(x):
    return np.asarray(x, np.float32).astype(np.float16)


def build_nc(n_b=B_PER_CORE, n_h=HEADS, with_l2=True, repeat=1):
    import concourse.bass as bass
    import concourse.mybir as mybir
    import concourse.tile as tile
    from concourse.masks import make_identity

    f16, f32, bf16 = mybir.dt.float16, mybir.dt.float32, mybir.dt.bfloat16
    nc = bass.Bass(target_bir_lowering=False, trn_type="TRN2")

    # ---- DRAM I/O (per core) ----
    xA_d = nc.dram_tensor("xA", [B_PER_CORE, 128, N], f16, kind="ExternalInput")
    xB_d = nc.dram_tensor("xB", [B_PER_CORE, 64, N], f16, kind="ExternalInput")
    WA_d = nc.dram_tensor("WA", [HEADS, 128, HID], f16, kind="ExternalInput")
    WB_d = nc.dram_tensor("WB", [HEADS, 64, HID], f16, kind="ExternalInput")
    gm_d = nc.dram_tensor("gm", [N, N], bf16, kind="ExternalInput")
    bhT_d = nc.dram_tensor("bhT", [HID, HEADS], f32, kind="ExternalInput")
    w2h_d = nc.dram_tensor("w2h", [N, C_OUT], f16, kind="ExternalInput")
    w2l_d = nc.dram_tensor("w2l", [N, C_OUT], f16, kind="ExternalInput")
    bo_d = nc.dram_tensor("bo", [C_OUT, 1], f32, kind="ExternalInput")
    outT_d = nc.dram_tensor("outT", [B_PER_CORE, C_OUT, N], f32,
                            kind="ExternalOutput")

    MM = nc.tensor.matmul
    EXP = mybir.ActivationFunctionType.Exp
    LRELU = mybir.ActivationFunctionType.Lrelu

    with tile.TileContext(nc) as tc:
        with (
            tc.tile_pool(name="const", bufs=1) as const,
            tc.tile_pool(name="hbuf", bufs=3) as hbuf,
            tc.tile_pool(name="abuf", bufs=4) as abuf,
            tc.tile_pool(name="atp", bufs=2) as atp,
            tc.tile_pool(name="x2p", bufs=2) as x2p,
            tc.tile_pool(name="smalls", bufs=24) as smalls,
            tc.tile_pool(name="ps_s", bufs=3, space="PSUM") as ps_s,
            tc.tile_pool(name="ps_x", bufs=1, space="PSUM") as ps_x,
        ):
            # ---- constants ----
            ident = const.tile([128, 128], bf16)
            make_identity(nc, ident)
            xA = const.tile([128, B_PER_CORE, N], f16)
            nc.gpsimd.dma_start(out=xA, in_=xA_d.rearrange("b p n -> p b n"))
            xB = const.tile([64, B_PER_CORE, N], f16)
            nc.gpsimd.dma_start(out=xB, in_=xB_d.rearrange("b p n -> p b n"))
            WA = const.tile([128, HEADS, HID], f16)
            nc.gpsimd.dma_start(out=WA, in_=WA_d.rearrange("h p d -> p h d"))
            WB = const.tile([64, HEADS, HID], f16)
            nc.gpsimd.dma_start(out=WB, in_=WB_d.rearrange("h p d -> p h d"))
            gm_t = const.tile([128, NBLK, N], bf16)
            gm_r = gm_d.rearrange("(i p) m -> p i m", p=128)
            for i in range(NBLK):
                nc.gpsimd.dma_start(out=gm_t[:, i:i + 1, :], in_=gm_r[:, i:i + 1, :])
            bhT = const.tile([HID, HEADS], f32)
            nc.gpsimd.dma_start(out=bhT, in_=bhT_d[:, :])
            w2h = const.tile([128, NBLK, C_OUT], f16)
            nc.gpsimd.dma_start(out=w2h, in_=w2h_d.rearrange("(k p) d -> p k d", p=128))
            w2l = const.tile([128, NBLK, C_OUT], f16)
            nc.gpsimd.dma_start(out=w2l, in_=w2l_d.rearrange("(k p) d -> p k d", p=128))
            bo = const.tile([C_OUT, 1], f32)
            nc.gpsimd.dma_start(out=bo, in_=bo_d[:, :])

            def softmax_blocks(sp_of_i, aT_int, smm_emit):
                """Emit per-n-block: gm-init + score matmuls + softmax + transpose.
                sp_of_i: fn(i) -> fresh psum tile; smm_emit: fn(sp, i) emits
                score matmuls accumulating into sp (start=False)."""
                for i in range(NBLK):
                    sp = sp_of_i(i)
                    for half in (slice(0, 512), slice(512, 1024)):
                        MM(sp[:, half], ident, gm_t[:, i, half],
                           start=True, stop=False)
                    smm_emit(sp, i)
                    negmax = smalls.tile([128, 1], f32, tag="negmax")
                    nc.vector.tensor_reduce(negmax, sp, axis=mybir.AxisListType.X,
                                            op=mybir.AluOpType.max, negate=True)
                    a_un = abuf.tile([128, N], f16, tag="a_un")
                    rowsum = smalls.tile([128, 1], f32, tag="rowsum")
                    nc.scalar.activation(a_un, sp, EXP, bias=negmax, scale=1.0,
                                         accum_out=rowsum)
                    recip = smalls.tile([128, 1], f32, tag="recip")
                    nc.vector.reciprocal(recip, rowsum)
                    a_n = abuf.tile([128, N], f16, tag="a_n")
                    nc.vector.tensor_scalar_mul(a_n, a_un, recip)
                    nc.sync.dma_start_transpose(
                        aT_int[:, :, 128 * i:128 * (i + 1)], a_n)

            def emit_h_phase(b, h):
                # ---- h = x @ W (split-fp16, K-stacked) -> hT in psum ----
                ph = ps_x.tile([128, N], f32, name="ph", tag="px")
                for half in (slice(0, 512), slice(512, 1024)):
                    MM(ph[:, half], WA[:, h, :], xA[:, b, half],
                       start=True, stop=False)
                    MM(ph[:, half], WB[:, h, :], xB[:, b, half],
                       start=False, stop=True)
                hT_H = hbuf.tile([128, N], f16, name="hT_H", tag="hT_H")
                nc.scalar.copy(hT_H, ph)
                hT_L = hbuf.tile([128, N], f16, name="hT_L", tag="hT_L")
                nc.vector.tensor_tensor(hT_L, ph, hT_H,
                                        mybir.AluOpType.subtract)
                hH_int = hbuf.tile([128, NBLK, 128], f16, name="hH_int", tag="hH_int")
                nc.sync.dma_start_transpose(hH_int, hT_H)
                hL_int = hbuf.tile([128, NBLK, 128], f16, name="hL_int", tag="hL_int")
                nc.sync.dma_start_transpose(hL_int, hT_L)
                return hT_H, hT_L, hH_int, hL_int

            bh_list = [(rep, b, h) for rep in range(repeat)
                       for b in range(n_b) for h in range(n_h)]
            x2HL = {}
            for rep in range(repeat):
                for b in range(n_b):
                    x2H = x2p.tile([128, HEADS, N], f16, name="x2H", tag="x2H")
                    x2L = x2p.tile([128, HEADS, N], f16, name="x2L", tag="x2L")
                    if n_h < HEADS:
                        nc.vector.memset(x2H, 0.0)
                        nc.vector.memset(x2L, 0.0)
                    x2HL[rep, b] = (x2H, x2L)
            hphase = emit_h_phase(*bh_list[0][1:])
            for idx, (rep, b, h) in enumerate(bh_list):
                x2H, x2L = x2HL[rep, b]
                hT_H, hT_L, hH_int, hL_int = hphase
                if idx + 1 < len(bh_list):
                    hphase = emit_h_phase(*bh_list[idx + 1][1:])
                if True:
                    # ---- scores + softmax + transpose ----
                    aT_int = atp.tile([128, NBLK, N], f16, name="aT", tag="aT")

                    def smm(sp, i):
                        Hi = hT_H[:, 128 * i:128 * (i + 1)]
                        Li = hT_L[:, 128 * i:128 * (i + 1)]
                        h0, h1 = slice(0, 512), slice(512, 1024)
                        MM(sp[:, h0], Hi, hT_H[:, h0], start=False, stop=False)
                        MM(sp[:, h1], Hi, hT_H[:, h1], start=False, stop=False)
                        MM(sp[:, h0], Hi, hT_L[:, h0], start=False, stop=False)
                        MM(sp[:, h1], Hi, hT_L[:, h1], start=False, stop=False)
                        MM(sp[:, h0], Li, hT_H[:, h0], start=False, stop=True)
                        MM(sp[:, h1], Li, hT_H[:, h1], start=False, stop=True)

                    softmax_blocks(
                        lambda i: ps_s.tile([128, N], f32, name="sp", tag="sp"),
                        aT_int, smm)

                    # ---- out2T = sum_j h_j^T @ aT_j  (2-pass fp16) ----
                    pav = ps_x.tile([128, N], f32, name="pav", tag="px")
                    h0, h1 = slice(0, 512), slice(512, 1024)
                    for j in range(NBLK):
                        MM(pav[:, h0], hH_int[:, j, :], aT_int[:, j, h0],
                           start=(j == 0), stop=False)
                        MM(pav[:, h1], hH_int[:, j, :], aT_int[:, j, h1],
                           start=(j == 0), stop=False)
                        MM(pav[:, h0], hL_int[:, j, :], aT_int[:, j, h0],
                           start=False, stop=(j == NBLK - 1))
                        MM(pav[:, h1], hL_int[:, j, :], aT_int[:, j, h1],
                           start=False, stop=(j == NBLK - 1))

                    # ---- x2 = leaky(out2T + bh), split to fp16 H/L ----
                    x2f = hbuf.tile([128, N], f32, tag="x2f")
                    nc.scalar.activation(x2f, pav, LRELU, bias=bhT[:, h:h + 1],
                                         scale=1.0, alpha=0.01)
                    nc.gpsimd.tensor_copy(x2H[:, h, :], x2f)
                    nc.gpsimd.tensor_tensor(x2L[:, h, :], x2f, x2H[:, h, :],
                                            mybir.AluOpType.subtract)

                if not with_l2:
                    nc.gpsimd.dma_start(out=outT_d[b], in_=x2f.bitcast(mybir.dt.float32)[0:64, :])
                    continue
                if h != n_h - 1:
                    continue
                # ======== layer 2 ========
                p2 = ps_x.tile([64, N], f32, name="p2", tag="px")
                h0, h1 = slice(0, 512), slice(512, 1024)
                for k in range(NBLK):
                    last = k == NBLK - 1
                    MM(p2[:, h0], w2h[:, k, :], x2H[:, k, h0], start=(k == 0), stop=False)
                    MM(p2[:, h1], w2h[:, k, :], x2H[:, k, h1], start=(k == 0), stop=False)
                    MM(p2[:, h0], w2h[:, k, :], x2L[:, k, h0], start=False, stop=False)
                    MM(p2[:, h1], w2h[:, k, :], x2L[:, k, h1], start=False, stop=False)
                    MM(p2[:, h0], w2l[:, k, :], x2H[:, k, h0], start=False, stop=last)
                    MM(p2[:, h1], w2l[:, k, :], x2H[:, k, h1], start=False, stop=last)
                h2T_H = hbuf.tile([64, N], f16, tag="h2T_H")
                nc.scalar.copy(h2T_H, p2)
                h2T_L = hbuf.tile([64, N], f16, tag="h2T_L")
                nc.vector.tensor_tensor(h2T_L, p2, h2T_H, mybir.AluOpType.subtract)
                h2_int = hbuf.tile([128, NBLK, 64], f16, tag="h2_int")
                nc.sync.dma_start_transpose(h2_int, h2T_H)

                a2T_int = atp.tile([128, NBLK, N], f16, tag="aT")

                def smm2(sp, i):
                    Hi = h2T_H[:, 128 * i:128 * (i + 1)]
                    Li = h2T_L[:, 128 * i:128 * (i + 1)]
                    h0, h1 = slice(0, 512), slice(512, 1024)
                    MM(sp[:, h0], Hi, h2T_H[:, h0], start=False, stop=False)
                    MM(sp[:, h1], Hi, h2T_H[:, h1], start=False, stop=False)
                    MM(sp[:, h0], Hi, h2T_L[:, h0], start=False, stop=False)
                    MM(sp[:, h1], Hi, h2T_L[:, h1], start=False, stop=False)
                    MM(sp[:, h0], Li, h2T_H[:, h0], start=False, stop=True)
                    MM(sp[:, h1], Li, h2T_H[:, h1], start=False, stop=True)

                softmax_blocks(
                    lambda i: ps_s.tile([128, N], f32, name="sp", tag="sp"),
                    a2T_int, smm2)

                po = ps_x.tile([64, N], f32, name="po", tag="px")
                h0, h1 = slice(0, 512), slice(512, 1024)
                for j in range(NBLK):
                    MM(po[:, h0], h2_int[:, j, :], a2T_int[:, j, h0],
                       start=(j == 0), stop=(j == NBLK - 1))
                    MM(po[:, h1], h2_int[:, j, :], a2T_int[:, j, h1],
                       start=(j == 0), stop=(j == NBLK - 1))
                out_sb = hbuf.tile([64, N], f32, tag="out_sb")
                nc.scalar.activation(out_sb, po, LRELU, bias=bo, scale=1.0,
                                     alpha=0.01)
                nc.gpsimd.dma_start(out=outT_d[b], in_=out_sb)

    orig = nc.to_json_bytes
    nc.to_json_bytes = lambda: _split_waits_json(orig())
    return nc


_NC_CACHE = None


def kernel(flow_x, graph, Wh, bh, W_out, b_out):
    global _NC_CACHE
    from concourse.bass_utils import run_bass_kernel_spmd

    flow_x = np.asarray(flow_x, np.float32)
    graph = np.asarray(graph, np.float32)
    Wh = np.asarray(Wh, np.float32)
    bh = np.asarray(bh, np.float32)
    W_out = np.asarray(W_out, np.float32)
    b_out = np.asarray(b_out, np.float32)

    import ml_dtypes

    # host-side prep (shared across cores)
    xT = np.ascontiguousarray(flow_x.transpose(0, 2, 1))        # [B, 64, N]
    xTH = _f16(xT)
    xTL = _f16(xT - xTH.astype(np.float32))
    xA_all = np.concatenate([xTH, xTH], axis=1)                  # [B, 128, N]
    WhH = _f16(Wh)
    WhL = _f16(Wh - WhH.astype(np.float32))
    WA = np.concatenate([WhH, WhL], axis=1)                      # [8, 128, 128]
    WB = WhH                                                     # [8, 64, 128]
    gm = ((graph - 1.0) * 1e16).astype(ml_dtypes.bfloat16)
    bhT = np.ascontiguousarray(bh.T)                             # [128, 8]
    w2h = _f16(W_out)
    w2l = _f16(W_out - w2h.astype(np.float32))
    bo = b_out.reshape(C_OUT, 1)

    in_maps = []
    for c in range(N_CORES):
        sl = slice(c * B_PER_CORE, (c + 1) * B_PER_CORE)
        in_maps.append({
            "xA": xA_all[sl], "xB": xTL[sl],
            "WA": WA, "WB": WB, "gm": gm, "bhT": bhT,
            "w2h": w2h, "w2l": w2l, "bo": bo,
        })

    if _NC_CACHE is None:
        _NC_CACHE = build_nc()
    res = run_bass_kernel_spmd(_NC_CACHE, in_maps, core_ids=list(range(N_CORES)))

    out = np.empty((B, N, C_OUT), np.float32)
    for c in range(N_CORES):
        o = res.results[c]["outT"]                               # [2, 64, N]
        for bl in range(B_PER_CORE):
            out[c * B_PER_CORE + bl] = o[bl].T
    return out



# revision 14
# speedup vs baseline: 1.0826x; 1.0826x over previous
"""Trainium2 Bass kernel for nn_GATNet (2-layer multi-head GAT).

Sharding: data-parallel over batch B=16 across 8 cores (2 batches/core);
graph and weights replicated. No collectives.

Per (batch, head): scores s = h@h.T are computed with split-fp16 (3 K-passes,
~fp32 accuracy at full PE rate); the adjacency mask is pre-added into PSUM via
an identity matmul of gm = (graph-1)*1e16; row-softmax uses a DVE
reduce_max(negate) + one ACT Exp pass (per-partition bias, fused row-sum);
attention matrices are transposed via the DMA xbar (fp16) for the a@h matmul,
which runs as a 2-pass fp16 split to keep h at ~fp32 precision (layer-2 scores
amplify any error in the layer-1 output by ~1e6).
"""
import json
import sys

import numpy as np

sys.path.insert(0, "/opt/trn_rl_repo")

B, N, C_IN, HID, HEADS, C_OUT = 16, 1024, 64, 128, 8, 64
N_CORES = 8
B_PER_CORE = B // N_CORES
NBLK = N // 128  # 8


def _split_waits_json(bir_bytes: bytes, max_waits: int = 1) -> bytes:
    """walrus in this toolchain accepts only one sync-wait per instruction;
    hoist extra waits onto preceding same-engine NoOps."""
    d = json.loads(bir_bytes)
    for fn in d.get("functions", []):
        for bb in fn.get("blocks", []):
            out = []
            for inst in bb.get("instructions", []):
                si = inst.get("sync_info") or {}
                waits = si.get("on_wait") or []
                if len(waits) > max_waits:
                    extra, si["on_wait"] = waits[:-max_waits], waits[-max_waits:]
                    for k in range(0, len(extra), max_waits):
                        out.append({
                            "opcode": "NoOp",
                            "name": f"{inst['name']}-wsplit{k}",
                            "engine": inst["engine"],
                            "ins": [],
                            "outs": [],
                            "sync_info": {"on_wait": extra[k:k + max_waits],
                                          "on_update": []},
                        })
                out.append(inst)
            bb["instructions"] = out
    return json.dumps(d).encode()


def _f16(x):
    return np.asarray(x, np.float32).astype(np.float16)


def build_nc(n_b=B_PER_CORE, n_h=HEADS, with_l2=True, repeat=1):
    import concourse.bass as bass
    import concourse.mybir as mybir
    import concourse.tile as tile
    from concourse.masks import make_identity

    f16, f32, bf16 = mybir.dt.float16, mybir.dt.float32, mybir.dt.bfloat16
    fp8e4, fp8e5 = mybir.dt.float8e4, mybir.dt.float8e5
    DR = mybir.MatmulPerfMode.DoubleRow
    nc = bass.Bass(target_bir_lowering=False, trn_type="TRN2")

    # ---- DRAM I/O (per core) ----
    xA_d = nc.dram_tensor("xA", [B_PER_CORE, 128, N], f16, kind="ExternalInput")
    xB_d = nc.dram_tensor("xB", [B_PER_CORE, 64, N], f16, kind="ExternalInput")
    WA_d = nc.dram_tensor("WA", [HEADS, 128, HID], f16, kind="ExternalInput")
    WB_d = nc.dram_tensor("WB", [HEADS, 64, HID], f16, kind="ExternalInput")
    gm_d = nc.dram_tensor("gm", [N, N], bf16, kind="ExternalInput")
    g8_d = nc.dram_tensor("g8", [128, NBLK + 1, N], fp8e4, kind="ExternalInput")
    idz_d = nc.dram_tensor("idz", [128, 2, 128], fp8e4, kind="ExternalInput")
    bhT_d = nc.dram_tensor("bhT", [HID, HEADS], f32, kind="ExternalInput")
    w2h_d = nc.dram_tensor("w2h", [N, 2 * C_OUT], f16, kind="ExternalInput")
    w2l_d = nc.dram_tensor("w2l", [N, 2 * C_OUT], f16, kind="ExternalInput")
    bo_d = nc.dram_tensor("bo", [C_OUT, 1], f32, kind="ExternalInput")
    outT_d = nc.dram_tensor("outT", [B_PER_CORE, C_OUT, N], f32,
                            kind="ExternalOutput")

    MM = nc.tensor.matmul
    EXP = mybir.ActivationFunctionType.Exp
    LRELU = mybir.ActivationFunctionType.Lrelu

    with tile.TileContext(nc) as tc:
        with (
            tc.tile_pool(name="const", bufs=1) as const,
            tc.tile_pool(name="hbuf", bufs=3) as hbuf,
            tc.tile_pool(name="l2buf", bufs=1) as l2buf,
            tc.tile_pool(name="abuf", bufs=3) as abuf,
            tc.tile_pool(name="atp", bufs=2) as atp,
            tc.tile_pool(name="x2p", bufs=2) as x2p,
            tc.tile_pool(name="smalls", bufs=24) as smalls,
            tc.tile_pool(name="ps_s", bufs=3, space="PSUM") as ps_s,
            tc.tile_pool(name="ps_x", bufs=1, space="PSUM") as ps_x,
        ):
            # ---- constants ----
            ident = const.tile([128, 128], bf16)
            make_identity(nc, ident)
            xA = const.tile([128, B_PER_CORE, N], f16)
            nc.gpsimd.dma_start(out=xA, in_=xA_d.rearrange("b p n -> p b n"))
            xB = const.tile([64, B_PER_CORE, N], f16)
            nc.gpsimd.dma_start(out=xB, in_=xB_d.rearrange("b p n -> p b n"))
            WA = const.tile([128, HEADS, HID], f16)
            nc.gpsimd.dma_start(out=WA, in_=WA_d.rearrange("h p d -> p h d"))
            WB = const.tile([64, HEADS, HID], f16)
            nc.gpsimd.dma_start(out=WB, in_=WB_d.rearrange("h p d -> p h d"))
            gm_t = const.tile([128, NBLK, N], bf16)
            gm_r = gm_d.rearrange("(i p) m -> p i m", p=128)
            for i in range(NBLK):
                nc.gpsimd.dma_start(out=gm_t[:, i:i + 1, :], in_=gm_r[:, i:i + 1, :])
            g8 = const.tile([128, NBLK + 1, N], fp8e4)
            nc.gpsimd.dma_start(out=g8, in_=g8_d[:, :, :])
            idz8 = const.tile([128, 2, 128], fp8e4)
            nc.gpsimd.dma_start(out=idz8, in_=idz_d[:, :, :])
            bhT = const.tile([HID, HEADS], f32)
            nc.gpsimd.dma_start(out=bhT, in_=bhT_d[:, :])
            w2h = const.tile([128, NBLK, 2 * C_OUT], f16)
            nc.gpsimd.dma_start(out=w2h, in_=w2h_d.rearrange("(k p) d -> p k d", p=128))
            w2l = const.tile([128, NBLK, 2 * C_OUT], f16)
            nc.gpsimd.dma_start(out=w2l, in_=w2l_d.rearrange("(k p) d -> p k d", p=128))
            bo = const.tile([C_OUT, 1], f32)
            nc.gpsimd.dma_start(out=bo, in_=bo_d[:, :])

            def softmax_blocks(sp_of_i, aT_int, smm_emit):
                """Emit per-n-block: score matmuls (incl. mask init) + softmax
                + transpose. sp_of_i: fn(i) -> fresh psum tile; smm_emit:
                fn(sp, i) emits mask-init + score matmuls into sp."""
                for i in range(NBLK):
                    sp = sp_of_i(i)
                    smm_emit(sp, i)
                    negmax = smalls.tile([128, 1], f32, tag="negmax")
                    nc.vector.tensor_reduce(negmax, sp, axis=mybir.AxisListType.X,
                                            op=mybir.AluOpType.max, negate=True)
                    a_un = abuf.tile([128, N], f16, tag="a_un")
                    rowsum = smalls.tile([128, 1], f32, tag="rowsum")
                    nc.scalar.activation(a_un, sp, EXP, bias=negmax, scale=1.0,
                                         accum_out=rowsum)
                    recip = smalls.tile([128, 1], f32, tag="recip")
                    nc.vector.reciprocal(recip, rowsum)
                    a_n = abuf.tile([128, N], f16, tag="a_n")
                    nc.vector.tensor_scalar_mul(a_n, a_un, recip)
                    nc.sync.dma_start_transpose(
                        aT_int[:, :, 128 * i:128 * (i + 1)], a_n)

            def emit_h_phase(b, h):
                # ---- h = x @ W (split-fp16, K-stacked) -> hT in psum ----
                ph = ps_x.tile([128, N], f32, name="ph", tag="px")
                for half in (slice(0, 512), slice(512, 1024)):
                    MM(ph[:, half], WA[:, h, :], xA[:, b, half],
                       start=True, stop=False)
                    MM(ph[:, half], WB[:, h, :], xB[:, b, half],
                       start=False, stop=True)
                hT_H = hbuf.tile([128, N], f16, name="hT_H", tag="hT_H")
                nc.scalar.copy(hT_H, ph)
                hT_L = hbuf.tile([128, N], f16, name="hT_L", tag="hT_L")
                nc.vector.tensor_tensor(hT_L, ph, hT_H,
                                        mybir.AluOpType.subtract)
                # fp8e5 pack (H, L, H) for DoubleRow cross-term matmuls
                p3 = hbuf.tile([128, 3, N], fp8e5, name="p3", tag="p3")
                nc.gpsimd.tensor_copy(p3[:, 0, :], hT_H)
                nc.gpsimd.tensor_copy(p3[:, 1, :], hT_L)
                nc.gpsimd.tensor_copy(p3[:, 2, :], hT_H)
                hH_int = hbuf.tile([128, NBLK, 128], f16, name="hH_int", tag="hH_int")
                nc.sync.dma_start_transpose(hH_int, hT_H)
                hL_int = hbuf.tile([128, NBLK, 128], f16, name="hL_int", tag="hL_int")
                nc.sync.dma_start_transpose(hL_int, hT_L)
                return hT_H, hT_L, hH_int, hL_int, p3

            bh_list = [(rep, b, h) for rep in range(repeat)
                       for b in range(n_b) for h in range(n_h)]
            x2HL = {}
            for rep in range(repeat):
                for b in range(n_b):
                    x2H = x2p.tile([128, HEADS, N], f16, name="x2H", tag="x2H")
                    x2L = x2p.tile([128, HEADS, N], f16, name="x2L", tag="x2L")
                    if n_h < HEADS:
                        nc.vector.memset(x2H, 0.0)
                        nc.vector.memset(x2L, 0.0)
                    x2HL[rep, b] = (x2H, x2L)
            hphase = emit_h_phase(*bh_list[0][1:])
            for idx, (rep, b, h) in enumerate(bh_list):
                x2H, x2L = x2HL[rep, b]
                hT_H, hT_L, hH_int, hL_int, p3 = hphase
                if idx + 1 < len(bh_list):
                    hphase = emit_h_phase(*bh_list[idx + 1][1:])
                if True:
                    # ---- scores + softmax + transpose ----
                    aT_int = atp.tile([128, NBLK, N], f16, name="aT", tag="aT")

                    def smm(sp, i):
                        Hi = hT_H[:, 128 * i:128 * (i + 1)]
                        Pi = p3[:, 0:2, 128 * i:128 * (i + 1)]
                        h0, h1 = slice(0, 512), slice(512, 1024)
                        # mask init: -57600 at masked entries via fp8 DR
                        # identity (slot1 of idz8 is zeros; g8 block i+1 is
                        # an ignored junk operand)
                        MM(sp[:, h0], idz8, g8[:, i:i + 2, h0],
                           start=True, stop=False, perf_mode=DR)
                        MM(sp[:, h1], idz8, g8[:, i:i + 2, h1],
                           start=True, stop=False, perf_mode=DR)
                        MM(sp[:, h0], Hi, hT_H[:, h0], start=False, stop=False)
                        MM(sp[:, h1], Hi, hT_H[:, h1], start=False, stop=False)
                        # cross terms H·L + L·H in one fp8e5 DoubleRow pass
                        MM(sp[:, h0], Pi, p3[:, 1:3, h0],
                           start=False, stop=True, perf_mode=DR)
                        MM(sp[:, h1], Pi, p3[:, 1:3, h1],
                           start=False, stop=True, perf_mode=DR)

                    softmax_blocks(
                        lambda i: ps_s.tile([128, N], f32, name="sp", tag="sp"),
                        aT_int, smm)

                    # ---- out2T = sum_j h_j^T @ aT_j  (2-pass fp16) ----
                    pav = ps_x.tile([128, N], f32, name="pav", tag="px")
                    h0, h1 = slice(0, 512), slice(512, 1024)
                    for j in range(NBLK):
                        MM(pav[:, h0], hH_int[:, j, :], aT_int[:, j, h0],
                           start=(j == 0), stop=False)
                        MM(pav[:, h1], hH_int[:, j, :], aT_int[:, j, h1],
                           start=(j == 0), stop=False)
                        MM(pav[:, h0], hL_int[:, j, :], aT_int[:, j, h0],
                           start=False, stop=(j == NBLK - 1))
                        MM(pav[:, h1], hL_int[:, j, :], aT_int[:, j, h1],
                           start=False, stop=(j == NBLK - 1))

                    # ---- x2 = leaky(out2T + bh), split to fp16 H/L ----
                    x2f = hbuf.tile([128, N], f32, tag="x2f")
                    nc.scalar.activation(x2f, pav, LRELU, bias=bhT[:, h:h + 1],
                                         scale=1.0, alpha=0.01)
                    nc.gpsimd.tensor_copy(x2H[:, h, :], x2f)
                    nc.gpsimd.tensor_tensor(x2L[:, h, :], x2f, x2H[:, h, :],
                                            mybir.AluOpType.subtract)

                if not with_l2:
                    nc.gpsimd.dma_start(out=outT_d[b], in_=x2f.bitcast(mybir.dt.float32)[0:64, :])
                    continue
                if h != n_h - 1:
                    continue
                # ======== layer 2 ========
                # p2 output partitions duplicated: rows 0:64 and 64:128 both
                # hold h2 (w2h/w2l are free-dim-duplicated), enabling
                # partition-aligned construction of the K-stacked tiles
                # h2s=[H;L] and h2sw=[L;H].
                p2 = ps_x.tile([128, N], f32, name="p2", tag="px")
                h0, h1 = slice(0, 512), slice(512, 1024)
                for k in range(NBLK):
                    last = k == NBLK - 1
                    MM(p2[:, h0], w2h[:, k, :], x2H[:, k, h0], start=(k == 0), stop=False)
                    MM(p2[:, h1], w2h[:, k, :], x2H[:, k, h1], start=(k == 0), stop=False)
                    MM(p2[:, h0], w2h[:, k, :], x2L[:, k, h0], start=False, stop=False)
                    MM(p2[:, h1], w2h[:, k, :], x2L[:, k, h1], start=False, stop=False)
                    MM(p2[:, h0], w2l[:, k, :], x2H[:, k, h0], start=False, stop=last)
                    MM(p2[:, h1], w2l[:, k, :], x2H[:, k, h1], start=False, stop=last)
                h2s = l2buf.tile([128, N], f16, tag="h2s")
                h2sw = l2buf.tile([128, N], f16, tag="h2sw")
                nc.scalar.copy(h2s[0:64, :], p2[0:64, :])
                nc.scalar.copy(h2sw[64:128, :], p2[64:128, :])
                nc.vector.tensor_tensor(h2sw[0:64, :], p2[0:64, :],
                                        h2s[0:64, :], mybir.AluOpType.subtract)
                nc.vector.tensor_tensor(h2s[64:128, :], p2[64:128, :],
                                        h2sw[64:128, :], mybir.AluOpType.subtract)
                h2_int = l2buf.tile([128, NBLK, 64], f16, tag="h2_int")
                nc.sync.dma_start_transpose(h2_int, h2s[0:64, :])

                a2T_int = atp.tile([128, NBLK, N], f16, tag="aT")

                def smm2(sp, i):
                    Hi = h2s[:, 128 * i:128 * (i + 1)]
                    h0, h1 = slice(0, 512), slice(512, 1024)
                    for half in (h0, h1):
                        MM(sp[:, half], ident, gm_t[:, i, half],
                           start=True, stop=False)
                    MM(sp[:, h0], Hi, h2s[:, h0], start=False, stop=False)
                    MM(sp[:, h1], Hi, h2s[:, h1], start=False, stop=False)
                    MM(sp[:, h0], Hi, h2sw[:, h0], start=False, stop=True)
                    MM(sp[:, h1], Hi, h2sw[:, h1], start=False, stop=True)

                softmax_blocks(
                    lambda i: ps_s.tile([128, N], f32, name="sp", tag="sp"),
                    a2T_int, smm2)

                po = ps_x.tile([64, N], f32, name="po", tag="px")
                h0, h1 = slice(0, 512), slice(512, 1024)
                for j in range(NBLK):
                    MM(po[:, h0], h2_int[:, j, :], a2T_int[:, j, h0],
                       start=(j == 0), stop=(j == NBLK - 1))
                    MM(po[:, h1], h2_int[:, j, :], a2T_int[:, j, h1],
                       start=(j == 0), stop=(j == NBLK - 1))
                out_sb = l2buf.tile([64, N], f32, tag="out_sb")
                nc.scalar.activation(out_sb, po, LRELU, bias=bo, scale=1.0,
                                     alpha=0.01)
                nc.gpsimd.dma_start(out=outT_d[b], in_=out_sb)

    orig = nc.to_json_bytes
    nc.to_json_bytes = lambda: _split_waits_json(orig())
    return nc


_NC_CACHE = None


def kernel(flow_x, graph, Wh, bh, W_out, b_out):
    global _NC_CACHE
    from concourse.bass_utils import run_bass_kernel_spmd

    flow_x = np.asarray(flow_x, np.float32)
    graph = np.asarray(graph, np.float32)
    Wh = np.asarray(Wh, np.float32)
    bh = np.asarray(bh, np.float32)
    W_out = np.asarray(W_out, np.float32)
    b_out = np.asarray(b_out, np.float32)

    import ml_dtypes

    # host-side prep (shared across cores)
    xT = np.ascontiguousarray(flow_x.transpose(0, 2, 1))        # [B, 64, N]
    xTH = _f16(xT)
    xTL = _f16(xT - xTH.astype(np.float32))
    xA_all = np.concatenate([xTH, xTH], axis=1)                  # [B, 128, N]
    WhH = _f16(Wh)
    WhL = _f16(Wh - WhH.astype(np.float32))
    WA = np.concatenate([WhH, WhL], axis=1)                      # [8, 128, 128]
    WB = WhH                                                     # [8, 64, 128]
    gm = ((graph - 1.0) * 1e16).astype(ml_dtypes.bfloat16)
    g8 = np.zeros((128, NBLK + 1, N), ml_dtypes.float8_e4m3)
    g8[:, :NBLK, :] = np.where(graph == 0.0, -240.0, 0.0).reshape(
        NBLK, 128, N).transpose(1, 0, 2).astype(ml_dtypes.float8_e4m3)
    idz = np.zeros((128, 2, 128), ml_dtypes.float8_e4m3)
    idz[:, 0, :] = (np.eye(128, dtype=np.float32) * 240.0).astype(
        ml_dtypes.float8_e4m3)
    bhT = np.ascontiguousarray(bh.T)                             # [128, 8]
    w2h1 = _f16(W_out)
    w2l1 = _f16(W_out - w2h1.astype(np.float32))
    w2h = np.concatenate([w2h1, w2h1], axis=1)                   # [1024, 128]
    w2l = np.concatenate([w2l1, w2l1], axis=1)
    bo = b_out.reshape(C_OUT, 1)

    in_maps = []
    for c in range(N_CORES):
        sl = slice(c * B_PER_CORE, (c + 1) * B_PER_CORE)
        in_maps.append({
            "xA": xA_all[sl], "xB": xTL[sl],
            "WA": WA, "WB": WB, "gm": gm, "g8": g8, "idz": idz, "bhT": bhT,
            "w2h": w2h, "w2l": w2l, "bo": bo,
        })

    if _NC_CACHE is None:
        _NC_CACHE = build_nc()
    res = run_bass_kernel_spmd(_NC_CACHE, in_maps, core_ids=list(range(N_CORES)))

    out = np.empty((B, N, C_OUT), np.float32)
    for c in range(N_CORES):
        o = res.results[c]["outT"]                               # [2, 64, N]
        for bl in range(B_PER_CORE):
            out[c * B_PER_CORE + bl] = o[bl].T
    return out

